# revision 26
# baseline (speedup 1.0000x reference)
"""Trainium2 Bass kernel for nn_EnhancedLocalAttention.

Reference semantics (B=4, L=4096, C=1024, H=16, D=64, WIN=256, step=128):
  qkv = x @ W_qkv + b_qkv -> q,k,v [B,H,L,D]
  overlapping windows n: tokens [n*128, n*128+256)
  per (b,h,n): S = (Q_win^T K_win)/8  (D x D, contracted over the 256 window
  tokens), P = softmax(S, axis=-1), O = P @ V_win^T  (D x W)
  regroup: rows of reshape(O, [256, 64]) laid at tokens n*256..n*256+255,
  slice to L -> only windows 0..15 survive; then @ W_out + b_out.

Sharding: 8 cores = (4 batches) x (2 window-halves of 8 windows each).
Each core consumes 9 x 128-token chunks and produces 2048 output rows.

Layout choices (all f16 on the PE):
- x is pre-transposed AND pre-cast to f16 on the host (x^T tiles land ready
  to use as the stationary operand): no on-chip x transposes, half the DMA.
- Weights are pre-cast to f16 on the host: halves the startup HBM read.
- Q,K projections are x-stationary (token-major output, as the S matmul
  needs); the V projection is W-stationary so its PSUM output is already
  V^T (channel-major, as the P@V matmul needs): the 72 PE transposes +
  copies of the V path disappear. V^T is computed in 3 token groups that
  slot into the DMA-gated early rounds.

Pipeline: rounds r=0..9 emit QKV for chunk r interleaved at unit granularity
with attention+out-proj for window r-2, so the in-order PE stream always has
dense independent matmul work between the serial softmax chains (keeps the
PE HAM clock-gate warm).
"""

import threading

import numpy as np

import concourse.bacc as bacc
import concourse.masks as masks
import concourse.mybir as mybir
import concourse.tile as tile
from concourse._compat import get_trn_type
from concourse.bass_utils import run_bass_kernel_spmd

F32 = mybir.dt.float32
F16 = mybir.dt.float16
EXP = mybir.ActivationFunctionType.Exp

B, L, C = 4, 4096, 1024
H, D, WIN, STEP = 16, 64, 256, 128
NCHUNK = 9            # 128-token chunks per core
NWIN = 8              # windows per core
TOK = NCHUNK * 128    # 1152 input tokens per core
OUT_ROWS = NWIN * 256 # 2048 output rows per core
VGROUPS = ((0, 512), (512, 512), (1024, 128))  # V^T token groups


def interleave(a, b):
    """Merge two unit lists proportionally (Bresenham)."""
    if not b:
        return list(a)
    if not a:
        return list(b)
    out = []
    ia = ib = 0
    while ia < len(a) or ib < len(b):
        if ib >= len(b) or (ia < len(a) and ia * len(b) <= ib * len(a)):
            out.append(a[ia]); ia += 1
        else:
            out.append(b[ib]); ib += 1
    return out


def build_program(with_bias=True):
    nc = bacc.Bacc(
        get_trn_type() or "TRN2",
        target_bir_lowering=False,
        debug=False,
        num_devices=8,
    )
    xt = nc.dram_tensor("xt", [C, TOK], F16, kind="ExternalInput")       # x^T
    wqk = nc.dram_tensor("wqk", [C, 2 * C], F16, kind="ExternalInput")   # W_qkv[:, :2C]
    wv = nc.dram_tensor("wv", [C, C], F16, kind="ExternalInput")         # W_qkv[:, 2C:]
    wout = nc.dram_tensor("wout", [C, C], F16, kind="ExternalInput")
    bqkv = nc.dram_tensor("bqkv", [3 * C], F32, kind="ExternalInput")
    bout = nc.dram_tensor("bout", [C], F32, kind="ExternalInput")
    out = nc.dram_tensor("out", [OUT_ROWS, C], F32, kind="ExternalOutput")

    from contextlib import ExitStack

    with tile.TileContext(nc) as tc, ExitStack() as ctx:
        pool = lambda name, bufs: ctx.enter_context(tc.tile_pool(name=name, bufs=bufs))
        xt_pool = pool("xt", 8)
        wq_pool = pool("wq", 8)
        wv_pool = pool("wv", 8)
        wo_pool = pool("wo", 8)
        const_pool = pool("const", 1)
        vt_pool = pool("vt", 1)
        q_pool = pool("q", 5)
        k_pool = pool("k", 5)
        at_pool = pool("at", 20)
        st_pool = pool("st", 10)
        yt_pool = pool("yt", 12)
        o_pool = pool("o", 3)
        ps = ctx.enter_context(tc.tile_pool(name="ps", bufs=8, space="PSUM"))

        # --- constants ---
        ones = const_pool.tile([1, 512], F16, tag="ones", name="ones")
        nc.vector.memset(ones[:], 1.0)

        # --- weight / input DMAs (order shapes the startup) ---
        # Pair xt/wqk per channel block: each arriving pair unlocks the
        # cb-major chunk-0/1 Q,K matmuls emitted below.
        xt_sb, wqk_sb = [], []
        for cb in range(8):
            t = xt_pool.tile([128, TOK], F16, tag="xt", name=f"xt{cb}")
            nc.gpsimd.dma_start(t[:], xt.ap()[cb * 128 : (cb + 1) * 128, :])
            xt_sb.append(t)
            w = wq_pool.tile([128, 2 * C], F16, tag="wqk", name=f"wqk{cb}")
            nc.gpsimd.dma_start(w[:], wqk.ap()[cb * 128 : (cb + 1) * 128, :])
            wqk_sb.append(w)
        wv_sb = []
        for cb in range(8):
            t = wv_pool.tile([128, C], F16, tag="wv", name=f"wv{cb}")
            nc.gpsimd.dma_start(t[:], wv.ap()[cb * 128 : (cb + 1) * 128, :])
            wv_sb.append(t)
        wo_sb = []
        for cb in range(8):
            t = wo_pool.tile([128, C], F16, tag="wo", name=f"wo{cb}")
            nc.gpsimd.dma_start(t[:], wout.ap()[cb * 128 : (cb + 1) * 128, :])
            wo_sb.append(t)
        bq_sb = const_pool.tile([1, 3 * C], F16, tag="bq", name="bq_sb")
        nc.gpsimd.dma_start(bq_sb[:], bqkv.ap().rearrange("(a f) -> a f", a=1))
        bo_sb = const_pool.tile([1, C], F16, tag="bo", name="bo_sb")
        nc.gpsimd.dma_start(bo_sb[:], bout.ap().rearrange("(a f) -> a f", a=1))
        idf16 = const_pool.tile([128, 128], F16, tag="idf16", name="idf16")
        masks.make_identity(nc, idf16[:])

        # PE warmup: dependency-free matmuls that run while the first weight
        # DMAs are in flight, so the HAM clock ramp (0.65 -> 2.4 GHz over
        # ~3us of continuous busy) completes before real work arrives.
        warm_ps = ps.tile([128, 512], F32, tag="ps", name="warm")
        for _ in range(24):
            nc.tensor.matmul(
                warm_ps[:], ones[:, 0:128], ones[:, 0:512], start=True, stop=True
            )

        # V^T lives in 8 persistent head-pair tiles [128 chan, TOK]
        vt_sb = [
            vt_pool.tile([128, TOK], F16, tag=f"vt{m}", name=f"vt{m}") for m in range(8)
        ]

        q_sb = [None] * NCHUNK
        k_sb = [None] * NCHUNK

        def qk_parts(r):
            """(alloc, u_mm(cb,ii) grid, fin) for chunk r's Q,K projection."""
            st = {}

            def u_alloc():
                st["pq"] = [
                    ps.tile([128, 512], F32, tag="ps", name=f"pq{i}") for i in range(4)
                ]

            def u_mm(cb, ii):
                def f():
                    for i in ii:
                        nc.tensor.matmul(
                            st["pq"][i][:],
                            xt_sb[cb][:, r * 128 : (r + 1) * 128],
                            wqk_sb[cb][:, i * 512 : (i + 1) * 512],
                            start=(cb == 0),
                            stop=(not with_bias and cb == 7),
                        )
                return f

            def u_fin():
                if with_bias:
                    for i in range(4):
                        nc.tensor.matmul(
                            st["pq"][i][:],
                            ones[:, 0:1],
                            bq_sb[:, i * 512 : (i + 1) * 512],
                            start=False,
                            stop=True,
                        )
                qt = q_pool.tile([128, C], F16, tag="q", name="qt")
                nc.vector.tensor_scalar_mul(qt[:, 0:512], st["pq"][0][:], 0.125)
                nc.vector.tensor_scalar_mul(qt[:, 512:1024], st["pq"][1][:], 0.125)
                q_sb[r] = qt
                kt = k_pool.tile([128, C], F16, tag="k", name="kt")
                nc.vector.tensor_copy(kt[:, 0:512], st["pq"][2][:])
                nc.vector.tensor_copy(kt[:, 512:1024], st["pq"][3][:])
                k_sb[r] = kt

            return u_alloc, u_mm, u_fin

        def qk_units(r):
            u_alloc, u_mm, u_fin = qk_parts(r)
            units = [u_alloc]
            for cb in range(8):
                units += [u_mm(cb, (0, 1)), u_mm(cb, (2, 3))]
            units += [u_fin]
            return units

        def head_units():
            """Chunks 0,1 Q,K emitted cb-major so each arriving (xt,wqk) DMA
            pair immediately releases both chunks' matmuls."""
            a0, mm0, f0 = qk_parts(0)
            a1, mm1, f1 = qk_parts(1)
            units = [a0, a1]
            for cb in range(8):
                units += [
                    mm0(cb, (0, 1)),
                    mm1(cb, (0, 1)),
                    mm0(cb, (2, 3)),
                    mm1(cb, (2, 3)),
                ]
            units += [f0, f1]
            return units

        def vgroup_units(g):
            """V^T for token group g, W-stationary: psum comes out [chan, tok]."""
            t0, tw = VGROUPS[g]

            def u_m(m):
                def f():
                    pv = ps.tile([128, tw], F32, tag="ps", name="pv")
                    for kb in range(8):
                        nc.tensor.matmul(
                            pv[:],
                            wv_sb[kb][:, m * 128 : (m + 1) * 128],
                            xt_sb[kb][:, t0 : t0 + tw],
                            start=(kb == 0),
                            stop=(not with_bias and kb == 7),
                        )
                    if with_bias:
                        # V^T bias: per-partition (out channel) constant
                        nc.tensor.matmul(
                            pv[:],
                            bq_sb[:, 2048 + m * 128 : 2048 + (m + 1) * 128],
                            ones[:, 0:tw],
                            start=False,
                            stop=True,
                        )
                    nc.vector.tensor_copy(vt_sb[m][:, t0 : t0 + tw], pv[:])
                return f

            return [u_m(m) for m in range(8)]

        def vgroup_units_kbmajor(g):
            """V^T group emitted kb-major: each arriving wv[kb] DMA releases
            one matmul for all 8 output blocks (head-of-kernel duty cycle).
            Holds all 8 PSUM banks, so only safe when nothing else is live."""
            t0, tw = VGROUPS[g]
            st = {}

            def u_alloc():
                st["pv"] = [
                    ps.tile([128, tw], F32, tag="ps", name=f"pv{m}") for m in range(8)
                ]

            def u_kb(kb):
                def f():
                    for m in range(8):
                        nc.tensor.matmul(
                            st["pv"][m][:],
                            wv_sb[kb][:, m * 128 : (m + 1) * 128],
                            xt_sb[kb][:, t0 : t0 + tw],
                            start=(kb == 0),
                            stop=(not with_bias and kb == 7),
                        )
                return f

            def u_fin():
                for m in range(8):
                    if with_bias:
                        nc.tensor.matmul(
                            st["pv"][m][:],
                            bq_sb[:, 2048 + m * 128 : 2048 + (m + 1) * 128],
                            ones[:, 0:tw],
                            start=False,
                            stop=True,
                        )
                    nc.vector.tensor_copy(vt_sb[m][:, t0 : t0 + tw], st["pv"][m][:])

            return [u_alloc] + [u_kb(kb) for kb in range(8)] + [u_fin]

        def make_window(r, tail=False):
            """Two unit lists for window r (chunks r, r+1): phase A (S +
            softmax + P^T transposes) and phase B (O matmuls + out-proj),
            emitted one round apart so the serial softmax chains of window
            r always overlap the dense O/out-proj matmuls of window r-1.
            tail=True streams the out-projection per head-pair (partial-K
            accumulation) to shorten the final drain."""
            yt = [None] * 8
            hps = [{} for _ in range(8)]

            def u_hp_s(hp):
                def f():
                    st = hps[hp]
                    s = ps.tile([128, 128], F32, tag="ps", name="s")
                    for rr, (b0, b1) in ((r, (True, False)), (r + 1, (False, True))):
                        nc.tensor.matmul(
                            s[:],
                            q_sb[rr][:, hp * 128 : (hp + 1) * 128],
                            k_sb[rr][:, hp * 128 : (hp + 1) * 128],
                            start=b0,
                            stop=b1,
                        )
                    p_exp = at_pool.tile([128, 64], F16, tag="p_exp", name="p_exp")
                    ssum = st_pool.tile([128, 1], F32, tag="ssum", name="ssum")
                    nc.scalar.activation(
                        p_exp[0:64, :], s[0:64, 0:64], EXP, accum_out=ssum[0:64, :]
                    )
                    nc.scalar.activation(
                        p_exp[64:128, :],
                        s[64:128, 64:128],
                        EXP,
                        accum_out=ssum[64:128, :],
                    )
                    rs = st_pool.tile([128, 1], F32, tag="rs", name="rs")
                    nc.vector.reciprocal(rs[:], ssum[:])
                    p_n = at_pool.tile([128, 64], F16, tag="p_n", name="p_n")
                    nc.vector.tensor_scalar_mul(p_n[:], p_exp[:], rs[:])
                    st["p_n"] = p_n
                return f

            def u_hp_t(hp):
                def f():
                    st = hps[hp]
                    p_n = st["p_n"]
                    ptp = ps.tile([128, 64], F16, tag="ps", name="ptp")
                    nc.tensor.transpose(
                        ptp[0:64, :], p_n[0:64, :], idf16[0:64, 0:64]
                    )
                    nc.tensor.transpose(
                        ptp[64:128, :], p_n[64:128, :], idf16[64:128, 64:128]
                    )
                    ptsb = at_pool.tile([128, 64], F16, tag="ptsb", name="ptsb")
                    nc.vector.tensor_copy(ptsb[:], ptp[:])
                    st["ptsb"] = ptsb
                return f

            def u_hp_o(hp):
                def f():
                    st = hps[hp]
                    h0 = 2 * hp
                    ptsb = st["ptsb"]
                    ypsum = ps.tile([128, 256], F32, tag="ps", name="ypsum")
                    for h, po in ((h0, 0), (h0 + 1, 64)):
                        rh = ptsb[po : po + 64, :]
                        for wq in range(4):
                            c0 = r * 128 + wq * 64
                            nc.tensor.matmul(
                                ypsum[po : po + 64, wq * 64 : (wq + 1) * 64],
                                vt_sb[h // 2][po : po + 64, c0 : c0 + 64],
                                rh,
                                start=True,
                                stop=True,
                            )
                    ytt = yt_pool.tile([128, 256], F16, tag="yt", name="ytt")
                    # Y^T[c, d*4+wq] = ypsum[c, wq*64+d]  (torch-unfold regroup)
                    nc.vector.tensor_copy(
                        ytt[:].rearrange("p (b a) -> p a b", a=4),
                        ypsum[:].rearrange("p (a b) -> p a b", a=4),
                    )
                    yt[hp] = ytt
                return f

            op_st = {}

            def u_op_alloc(th):
                def f():
                    op_st[th] = [
                        ps.tile([128, 512], F32, tag="ps", name=f"pom{th}{i}")
                        for i in range(2)
                    ]
                return f

            def u_op_mm(th, cb):
                def f():
                    for mi in range(2):
                        nc.tensor.matmul(
                            op_st[th][mi][:],
                            yt[cb][:, th * 128 : (th + 1) * 128],
                            wo_sb[cb][:, mi * 512 : (mi + 1) * 512],
                            start=(cb == 0),
                            stop=(not with_bias and cb == 7),
                        )
                return f

            def u_op_fin(th):
                def f():
                    po_m = op_st[th]
                    if with_bias:
                        for mi in range(2):
                            nc.tensor.matmul(
                                po_m[mi][:],
                                ones[:, 0:1],
                                bo_sb[:, mi * 512 : (mi + 1) * 512],
                                start=False,
                                stop=True,
                            )
                    ot = o_pool.tile([128, C], F16, tag="o", name="ot")
                    row = r * 256 + th * 128
                    nc.vector.tensor_copy(ot[:, 0:512], po_m[0][:])
                    nc.gpsimd.dma_start(out.ap()[row : row + 128, 0:512], ot[:, 0:512])
                    nc.vector.tensor_copy(ot[:, 512:1024], po_m[1][:])
                    nc.gpsimd.dma_start(
                        out.ap()[row : row + 128, 512:1024], ot[:, 512:1024]
                    )
                return f

            def u_op(th):
                def f():
                    u_op_alloc(th)()
                    for cb in range(8):
                        u_op_mm(th, cb)()
                    u_op_fin(th)()
                return f

            units_a = [u_hp_s(0), u_hp_s(1), u_hp_t(0), u_hp_s(2), u_hp_t(1)]
            for hp in range(3, 8):
                units_a += [u_hp_s(hp), u_hp_t(hp - 1)]
            units_a += [u_hp_t(7)]
            if not tail:
                units_b = [u_hp_o(hp) for hp in range(8)]
                units_b += [u_op(0), u_op(1)]
            else:
                # stream the out-projection per head-pair as yt tiles land
                units_b = [u_op_alloc(0), u_op_alloc(1)]
                for hp in range(8):
                    units_b += [u_hp_o(hp)]
                    if hp >= 1:
                        units_b += [u_op_mm(0, hp - 1), u_op_mm(1, hp - 1)]
                units_b += [u_op_mm(0, 7), u_op_mm(1, 7)]
                units_b += [u_op_fin(0), u_op_fin(1)]
            return units_a, units_b

        # Head: chunks 0,1 cb-major (tracks the paired DMA arrivals), then
        # V^T group 0 kb-major (tracks the wv DMA arrivals).
        for u in head_units():
            u()
        for u in vgroup_units_kbmajor(0):
            u()
        # Steady rounds r=2..10: QKV chunk r, window r-2 phase A (softmax),
        # window r-3 phase B (O + out-proj); V groups 1,2 woven in.
        win_a = {}
        win_b = {}
        for r in range(NWIN):
            win_a[r], win_b[r] = make_window(r, tail=(r == NWIN - 1))
        vg_by_round = {3: 1, 5: 2}
        for r in range(2, NCHUNK + 2):
            units = qk_units(r) if r < NCHUNK else []
            if r in vg_by_round:
                units = interleave(units, vgroup_units(vg_by_round[r]))
            dense = interleave(units, win_b.get(r - 3, []))
            for u in interleave(dense, win_a.get(r - 2, [])):
                u()

    nc.compile()
    return nc


_CACHE = {}
_LOCK = threading.Lock()


def _get_program(with_bias=True):
    key = f"nc_bias{with_bias}"
    with _LOCK:
        if key not in _CACHE:
            _CACHE[key] = build_program(with_bias=with_bias)
        return _CACHE[key]


def _in_maps(x, W_qkv, b_qkv, W_out, b_out):
    wqk16 = np.ascontiguousarray(W_qkv[:, : 2 * C]).astype(np.float16)
    wv16 = np.ascontiguousarray(W_qkv[:, 2 * C :]).astype(np.float16)
    wo16 = W_out.astype(np.float16)
    maps = []
    for cid in range(8):
        b, half = cid // 2, cid % 2
        t0 = half * NWIN * STEP
        xt16 = np.ascontiguousarray(x[b, t0 : t0 + TOK, :].T).astype(np.float16)
        maps.append(
            {
                "xt": xt16,
                "wqk": wqk16,
                "wv": wv16,
                "wout": wo16,
                "bqkv": b_qkv,
                "bout": b_out,
            }
        )
    return maps


def kernel(x, W_qkv, b_qkv, W_out, b_out):
    x = np.asarray(x, dtype=np.float32)
    W_qkv = np.asarray(W_qkv, dtype=np.float32)
    b_qkv = np.asarray(b_qkv, dtype=np.float32)
    W_out = np.asarray(W_out, dtype=np.float32)
    b_out = np.asarray(b_out, dtype=np.float32)

    with_bias = bool(np.any(b_qkv)) or bool(np.any(b_out))
    nc = _get_program(with_bias=with_bias)
    res = run_bass_kernel_spmd(
        nc, _in_maps(x, W_qkv, b_qkv, W_out, b_out), core_ids=list(range(8))
    )
    out_full = np.empty((B, L, C), dtype=np.float32)
    for cid in range(8):
        b, half = cid // 2, cid % 2
        out_full[b, half * OUT_ROWS : (half + 1) * OUT_ROWS, :] = res.results[cid][
            "out"
        ]
    return out_full


# revision 30
# speedup vs baseline: 1.0398x; 1.0398x over previous
"""Trainium2 Bass kernel for nn_EnhancedLocalAttention.

Reference semantics (B=4, L=4096, C=1024, H=16, D=64, WIN=256, step=128):
  qkv = x @ W_qkv + b_qkv -> q,k,v [B,H,L,D]
  overlapping windows n: tokens [n*128, n*128+256)
  per (b,h,n): S = (Q_win^T K_win)/8  (D x D, contracted over the 256 window
  tokens), P = softmax(S, axis=-1), O = P @ V_win^T  (D x W)
  regroup: rows of reshape(O, [256, 64]) laid at tokens n*256..n*256+255,
  slice to L -> only windows 0..15 survive; then @ W_out + b_out.

Sharding: 8 cores = (4 batches) x (2 window-halves of 8 windows each).
Each core consumes 9 x 128-token chunks and produces 2048 output rows.

Layout choices (all f16 on the PE):
- x is pre-transposed AND pre-cast to f16 on the host (x^T tiles land ready
  to use as the stationary operand): no on-chip x transposes, half the DMA.
- Weights are pre-cast to f16 on the host: halves the startup HBM read.
- Q,K projections are x-stationary (token-major output, as the S matmul
  needs); the V projection is W-stationary so its PSUM output is already
  V^T (channel-major, as the P@V matmul needs): the 72 PE transposes +
  copies of the V path disappear. V^T is computed in 3 token groups that
  slot into the DMA-gated early rounds.

Pipeline: rounds r=0..9 emit QKV for chunk r interleaved at unit granularity
with attention+out-proj for window r-2, so the in-order PE stream always has
dense independent matmul work between the serial softmax chains (keeps the
PE HAM clock-gate warm).
"""

import threading

import numpy as np

import concourse.bacc as bacc
import concourse.masks as masks
import concourse.mybir as mybir
import concourse.tile as tile
from concourse._compat import get_trn_type
from concourse.bass_utils import run_bass_kernel_spmd

F32 = mybir.dt.float32
F16 = mybir.dt.float16
EXP = mybir.ActivationFunctionType.Exp

B, L, C = 4, 4096, 1024
H, D, WIN, STEP = 16, 64, 256, 128
NCHUNK = 9            # 128-token chunks per core
NWIN = 8              # windows per core
TOK = NCHUNK * 128    # 1152 input tokens per core
OUT_ROWS = NWIN * 256 # 2048 output rows per core
VGROUPS = ((0, 512), (512, 512), (1024, 128))  # V^T token groups


def interleave(a, b):
    """Merge two unit lists proportionally (Bresenham)."""
    if not b:
        return list(a)
    if not a:
        return list(b)
    out = []
    ia = ib = 0
    while ia < len(a) or ib < len(b):
        if ib >= len(b) or (ia < len(a) and ia * len(b) <= ib * len(a)):
            out.append(a[ia]); ia += 1
        else:
            out.append(b[ib]); ib += 1
    return out


def build_program(with_bias=True):
    nc = bacc.Bacc(
        get_trn_type() or "TRN2",
        target_bir_lowering=False,
        debug=False,
        num_devices=8,
    )
    xt = nc.dram_tensor("xt", [C, TOK], F16, kind="ExternalInput")       # x^T
    wqk = nc.dram_tensor("wqk", [C, 2 * C], F16, kind="ExternalInput")   # W_qkv[:, :2C]
    wv = nc.dram_tensor("wv", [C, C], F16, kind="ExternalInput")         # W_qkv[:, 2C:]
    wout = nc.dram_tensor("wout", [C, C], F16, kind="ExternalInput")
    bqkv = nc.dram_tensor("bqkv", [3 * C], F32, kind="ExternalInput")
    bout = nc.dram_tensor("bout", [C], F32, kind="ExternalInput")
    out = nc.dram_tensor("out", [OUT_ROWS, C], F16, kind="ExternalOutput")

    from contextlib import ExitStack

    with tile.TileContext(nc) as tc, ExitStack() as ctx:
        pool = lambda name, bufs: ctx.enter_context(tc.tile_pool(name=name, bufs=bufs))
        xt_pool = pool("xt", 8)
        wq_pool = pool("wq", 8)
        wv_pool = pool("wv", 8)
        wo_pool = pool("wo", 8)
        const_pool = pool("const", 1)
        vt_pool = pool("vt", 1)
        q_pool = pool("q", 5)
        k_pool = pool("k", 5)
        at_pool = pool("at", 20)
        st_pool = pool("st", 10)
        yt_pool = pool("yt", 12)
        o_pool = pool("o", 3)
        ps = ctx.enter_context(tc.tile_pool(name="ps", bufs=8, space="PSUM"))

        # --- constants ---
        ones = const_pool.tile([1, 512], F16, tag="ones", name="ones")
        nc.vector.memset(ones[:], 1.0)

        # --- weight / input DMAs (order shapes the startup) ---
        # Pair xt/wqk per channel block: each arriving pair unlocks the
        # cb-major chunk-0/1 Q,K matmuls emitted below.
        xt_sb, wqk_sb = [], []
        for cb in range(8):
            t = xt_pool.tile([128, TOK], F16, tag="xt", name=f"xt{cb}")
            nc.gpsimd.dma_start(t[:], xt.ap()[cb * 128 : (cb + 1) * 128, :])
            xt_sb.append(t)
            w = wq_pool.tile([128, 2 * C], F16, tag="wqk", name=f"wqk{cb}")
            nc.gpsimd.dma_start(w[:], wqk.ap()[cb * 128 : (cb + 1) * 128, :])
            wqk_sb.append(w)
        wv_sb = []
        for cb in range(8):
            t = wv_pool.tile([128, C], F16, tag="wv", name=f"wv{cb}")
            nc.gpsimd.dma_start(t[:], wv.ap()[cb * 128 : (cb + 1) * 128, :])
            wv_sb.append(t)
        wo_sb = []
        for cb in range(8):
            t = wo_pool.tile([128, C], F16, tag="wo", name=f"wo{cb}")
            nc.gpsimd.dma_start(t[:], wout.ap()[cb * 128 : (cb + 1) * 128, :])
            wo_sb.append(t)
        bq_sb = const_pool.tile([1, 3 * C], F16, tag="bq", name="bq_sb")
        nc.gpsimd.dma_start(bq_sb[:], bqkv.ap().rearrange("(a f) -> a f", a=1))
        bo_sb = const_pool.tile([1, C], F16, tag="bo", name="bo_sb")
        nc.gpsimd.dma_start(bo_sb[:], bout.ap().rearrange("(a f) -> a f", a=1))
        idf16 = const_pool.tile([128, 128], F16, tag="idf16", name="idf16")
        masks.make_identity(nc, idf16[:])

        # PE warmup: dependency-free matmuls that run while the first weight
        # DMAs are in flight, so the HAM clock ramp (0.65 -> 2.4 GHz over
        # ~3us of continuous busy) completes before real work arrives.
        warm_ps = ps.tile([128, 512], F32, tag="ps", name="warm")
        for _ in range(10):
            nc.tensor.matmul(
                warm_ps[:], ones[:, 0:128], ones[:, 0:512], start=True, stop=True
            )

        # V^T lives in 8 persistent head-pair tiles [128 chan, TOK]
        vt_sb = [
            vt_pool.tile([128, TOK], F16, tag=f"vt{m}", name=f"vt{m}") for m in range(8)
        ]

        q_sb = [None] * NCHUNK
        k_sb = [None] * NCHUNK

        def qk_parts(r):
            """(alloc, u_mm(cb,ii) grid, fin) for chunk r's Q,K projection."""
            st = {}

            def u_alloc():
                st["pq"] = [
                    ps.tile([128, 512], F32, tag="ps", name=f"pq{i}") for i in range(4)
                ]

            def u_mm(cb, ii):
                def f():
                    for i in ii:
                        nc.tensor.matmul(
                            st["pq"][i][:],
                            xt_sb[cb][:, r * 128 : (r + 1) * 128],
                            wqk_sb[cb][:, i * 512 : (i + 1) * 512],
                            start=(cb == 0),
                            stop=(not with_bias and cb == 7),
                        )
                return f

            def u_fin():
                if with_bias:
                    for i in range(4):
                        nc.tensor.matmul(
                            st["pq"][i][:],
                            ones[:, 0:1],
                            bq_sb[:, i * 512 : (i + 1) * 512],
                            start=False,
                            stop=True,
                        )
                qt = q_pool.tile([128, C], F16, tag="q", name="qt")
                nc.vector.tensor_scalar_mul(qt[:, 0:512], st["pq"][0][:], 0.125)
                nc.vector.tensor_scalar_mul(qt[:, 512:1024], st["pq"][1][:], 0.125)
                q_sb[r] = qt
                kt = k_pool.tile([128, C], F16, tag="k", name="kt")
                nc.vector.tensor_copy(kt[:, 0:512], st["pq"][2][:])
                nc.vector.tensor_copy(kt[:, 512:1024], st["pq"][3][:])
                k_sb[r] = kt

            return u_alloc, u_mm, u_fin

        def qk_units(r):
            u_alloc, u_mm, u_fin = qk_parts(r)
            units = [u_alloc]
            for cb in range(8):
                units += [u_mm(cb, (0, 1)), u_mm(cb, (2, 3))]
            units += [u_fin]
            return units

        def head_units():
            """Chunks 0,1 Q,K emitted cb-major so each arriving (xt,wqk) DMA
            pair immediately releases both chunks' matmuls."""
            a0, mm0, f0 = qk_parts(0)
            a1, mm1, f1 = qk_parts(1)
            units = [a0, a1]
            for cb in range(8):
                units += [
                    mm0(cb, (0, 1)),
                    mm1(cb, (0, 1)),
                    mm0(cb, (2, 3)),
                    mm1(cb, (2, 3)),
                ]
            units += [f0, f1]
            return units

        def vgroup_units(g):
            """V^T for token group g, W-stationary: psum comes out [chan, tok]."""
            t0, tw = VGROUPS[g]

            def u_m(m):
                def f():
                    pv = ps.tile([128, tw], F32, tag="ps", name="pv")
                    for kb in range(8):
                        nc.tensor.matmul(
                            pv[:],
                            wv_sb[kb][:, m * 128 : (m + 1) * 128],
                            xt_sb[kb][:, t0 : t0 + tw],
                            start=(kb == 0),
                            stop=(not with_bias and kb == 7),
                        )
                    if with_bias:
                        # V^T bias: per-partition (out channel) constant
                        nc.tensor.matmul(
                            pv[:],
                            bq_sb[:, 2048 + m * 128 : 2048 + (m + 1) * 128],
                            ones[:, 0:tw],
                            start=False,
                            stop=True,
                        )
                    nc.vector.tensor_copy(vt_sb[m][:, t0 : t0 + tw], pv[:])
                return f

            return [u_m(m) for m in range(8)]

        def vgroup_units_kbmajor(g):
            """V^T group emitted kb-major: each arriving wv[kb] DMA releases
            one matmul for all 8 output blocks (head-of-kernel duty cycle).
            Holds all 8 PSUM banks, so only safe when nothing else is live."""
            t0, tw = VGROUPS[g]
            st = {}

            def u_alloc():
                st["pv"] = [
                    ps.tile([128, tw], F32, tag="ps", name=f"pv{m}") for m in range(8)
                ]

            def u_kb(kb):
                def f():
                    for m in range(8):
                        nc.tensor.matmul(
                            st["pv"][m][:],
                            wv_sb[kb][:, m * 128 : (m + 1) * 128],
                            xt_sb[kb][:, t0 : t0 + tw],
                            start=(kb == 0),
                            stop=(not with_bias and kb == 7),
                        )
                return f

            def u_fin():
                for m in range(8):
                    if with_bias:
                        nc.tensor.matmul(
                            st["pv"][m][:],
                            bq_sb[:, 2048 + m * 128 : 2048 + (m + 1) * 128],
                            ones[:, 0:tw],
                            start=False,
                            stop=True,
                        )
                    nc.vector.tensor_copy(vt_sb[m][:, t0 : t0 + tw], st["pv"][m][:])

            return [u_alloc] + [u_kb(kb) for kb in range(8)] + [u_fin]

        def make_window(r, tail=False):
            """Two unit lists for window r (chunks r, r+1): phase A (S +
            softmax + P^T transposes) and phase B (O matmuls + out-proj),
            emitted one round apart so the serial softmax chains of window
            r always overlap the dense O/out-proj matmuls of window r-1.
            tail=True streams the out-projection per head-pair (partial-K
            accumulation) to shorten the final drain."""
            yt = [None] * 8
            hps = [{} for _ in range(8)]

            def u_hp_s(hp):
                def f():
                    st = hps[hp]
                    s = ps.tile([128, 128], F32, tag="ps", name="s")
                    for rr, (b0, b1) in ((r, (True, False)), (r + 1, (False, True))):
                        nc.tensor.matmul(
                            s[:],
                            q_sb[rr][:, hp * 128 : (hp + 1) * 128],
                            k_sb[rr][:, hp * 128 : (hp + 1) * 128],
                            start=b0,
                            stop=b1,
                        )
                    p_exp = at_pool.tile([128, 64], F16, tag="p_exp", name="p_exp")
                    ssum = st_pool.tile([128, 1], F32, tag="ssum", name="ssum")
                    nc.scalar.activation(
                        p_exp[0:64, :], s[0:64, 0:64], EXP, accum_out=ssum[0:64, :]
                    )
                    nc.scalar.activation(
                        p_exp[64:128, :],
                        s[64:128, 64:128],
                        EXP,
                        accum_out=ssum[64:128, :],
                    )
                    rs = st_pool.tile([128, 1], F32, tag="rs", name="rs")
                    nc.vector.reciprocal(rs[:], ssum[:])
                    p_n = at_pool.tile([128, 64], F16, tag="p_n", name="p_n")
                    nc.vector.tensor_scalar_mul(p_n[:], p_exp[:], rs[:])
                    st["p_n"] = p_n
                return f

            def u_hp_t(hp):
                def f():
                    st = hps[hp]
                    p_n = st["p_n"]
                    ptp = ps.tile([128, 64], F16, tag="ps", name="ptp")
                    nc.tensor.transpose(
                        ptp[0:64, :], p_n[0:64, :], idf16[0:64, 0:64]
                    )
                    nc.tensor.transpose(
                        ptp[64:128, :], p_n[64:128, :], idf16[64:128, 64:128]
                    )
                    ptsb = at_pool.tile([128, 64], F16, tag="ptsb", name="ptsb")
                    nc.vector.tensor_copy(ptsb[:], ptp[:])
                    st["ptsb"] = ptsb
                return f

            def u_hp_o(hp):
                def f():
                    st = hps[hp]
                    h0 = 2 * hp
                    ptsb = st["ptsb"]
                    ypsum = ps.tile([128, 256], F32, tag="ps", name="ypsum")
                    for h, po in ((h0, 0), (h0 + 1, 64)):
                        rh = ptsb[po : po + 64, :]
                        for wq in range(4):
                            c0 = r * 128 + wq * 64
                            nc.tensor.matmul(
                                ypsum[po : po + 64, wq * 64 : (wq + 1) * 64],
                                vt_sb[h // 2][po : po + 64, c0 : c0 + 64],
                                rh,
                                start=True,
                                stop=True,
                            )
                    ytt = yt_pool.tile([128, 256], F16, tag="yt", name="ytt")
                    # Y^T[c, d*4+wq] = ypsum[c, wq*64+d]  (torch-unfold regroup)
                    nc.vector.tensor_copy(
                        ytt[:].rearrange("p (b a) -> p a b", a=4),
                        ypsum[:].rearrange("p (a b) -> p a b", a=4),
                    )
                    yt[hp] = ytt
                return f

            op_st = {}

            def u_op_alloc(th):
                def f():
                    op_st[th] = [
                        ps.tile([128, 512], F32, tag="ps", name=f"pom{th}{i}")
                        for i in range(2)
                    ]
                return f

            def u_op_mm(th, cb):
                def f():
                    for mi in range(2):
                        nc.tensor.matmul(
                            op_st[th][mi][:],
                            yt[cb][:, th * 128 : (th + 1) * 128],
                            wo_sb[cb][:, mi * 512 : (mi + 1) * 512],
                            start=(cb == 0),
                            stop=(not with_bias and cb == 7),
                        )
                return f

            def u_op_fin(th):
                def f():
                    po_m = op_st[th]
                    if with_bias:
                        for mi in range(2):
                            nc.tensor.matmul(
                                po_m[mi][:],
                                ones[:, 0:1],
                                bo_sb[:, mi * 512 : (mi + 1) * 512],
                                start=False,
                                stop=True,
                            )
                    ot = o_pool.tile([128, C], F16, tag="o", name="ot")
                    row = r * 256 + th * 128
                    nc.vector.tensor_copy(ot[:, 0:512], po_m[0][:])
                    nc.vector.tensor_copy(ot[:, 512:1024], po_m[1][:])
                    nc.sync.dma_start(out.ap()[row : row + 128, :], ot[:])
                return f

            def u_op(th):
                def f():
                    u_op_alloc(th)()
                    for cb in range(8):
                        u_op_mm(th, cb)()
                    u_op_fin(th)()
                return f

            units_a = [u_hp_s(0), u_hp_s(1), u_hp_t(0), u_hp_s(2), u_hp_t(1)]
            for hp in range(3, 8):
                units_a += [u_hp_s(hp), u_hp_t(hp - 1)]
            units_a += [u_hp_t(7)]
            if not tail:
                units_b = [u_hp_o(hp) for hp in range(8)]
                units_b += [u_op(0), u_op(1)]
            else:
                # stream the out-projection per head-pair as yt tiles land
                units_b = [u_op_alloc(0), u_op_alloc(1)]
                for hp in range(8):
                    units_b += [u_hp_o(hp)]
                    if hp >= 1:
                        units_b += [u_op_mm(0, hp - 1), u_op_mm(1, hp - 1)]
                units_b += [u_op_mm(0, 7), u_op_mm(1, 7)]
                units_b += [u_op_fin(0), u_op_fin(1)]
            return units_a, units_b

        # Head: chunks 0,1 cb-major (tracks the paired DMA arrivals), then
        # V^T group 0 kb-major (tracks the wv DMA arrivals).
        for u in head_units():
            u()
        for u in vgroup_units_kbmajor(0):
            u()
        # Steady rounds r=2..10: QKV chunk r, window r-2 phase A (softmax),
        # window r-3 phase B (O + out-proj); V groups 1,2 woven in.
        win_a = {}
        win_b = {}
        for r in range(NWIN):
            win_a[r], win_b[r] = make_window(r, tail=(r == NWIN - 1))
        vg_by_round = {3: 1, 5: 2}
        for r in range(2, NCHUNK + 2):
            units = qk_units(r) if r < NCHUNK else []
            if r in vg_by_round:
                units = interleave(units, vgroup_units(vg_by_round[r]))
            dense = interleave(units, win_b.get(r - 3, []))
            for u in interleave(dense, win_a.get(r - 2, [])):
                u()

    nc.compile()
    return nc


_CACHE = {}
_LOCK = threading.Lock()


def _get_program(with_bias=True):
    key = f"nc_bias{with_bias}"
    with _LOCK:
        if key not in _CACHE:
            _CACHE[key] = build_program(with_bias=with_bias)
        return _CACHE[key]


def _in_maps(x, W_qkv, b_qkv, W_out, b_out):
    wqk16 = np.ascontiguousarray(W_qkv[:, : 2 * C]).astype(np.float16)
    wv16 = np.ascontiguousarray(W_qkv[:, 2 * C :]).astype(np.float16)
    wo16 = W_out.astype(np.float16)
    maps = []
    for cid in range(8):
        b, half = cid // 2, cid % 2
        t0 = half * NWIN * STEP
        xt16 = np.ascontiguousarray(x[b, t0 : t0 + TOK, :].T).astype(np.float16)
        maps.append(
            {
                "xt": xt16,
                "wqk": wqk16,
                "wv": wv16,
                "wout": wo16,
                "bqkv": b_qkv,
                "bout": b_out,
            }
        )
    return maps


def kernel(x, W_qkv, b_qkv, W_out, b_out):
    x = np.asarray(x, dtype=np.float32)
    W_qkv = np.asarray(W_qkv, dtype=np.float32)
    b_qkv = np.asarray(b_qkv, dtype=np.float32)
    W_out = np.asarray(W_out, dtype=np.float32)
    b_out = np.asarray(b_out, dtype=np.float32)

    with_bias = bool(np.any(b_qkv)) or bool(np.any(b_out))
    nc = _get_program(with_bias=with_bias)
    res = run_bass_kernel_spmd(
        nc, _in_maps(x, W_qkv, b_qkv, W_out, b_out), core_ids=list(range(8))
    )
    out_full = np.empty((B, L, C), dtype=np.float32)
    for cid in range(8):
        b, half = cid // 2, cid % 2
        out_full[b, half * OUT_ROWS : (half + 1) * OUT_ROWS, :] = res.results[cid][
            "out"
        ].astype(np.float32)
    return out_full
